# revision 17
# baseline (speedup 1.0000x reference)
"""Multi-head attention (RoPE + causal softmax) Bass kernel for 8 TRN2 cores.

Problem: B=2, S=2048, D=1024, H=16 heads, d_k=64.
Sharding: data-parallel over batch (2) x tensor-parallel over heads (4 groups
of 4 heads).  Core c handles batch c//4, heads [4*(c%4), 4*(c%4)+4).
Each core computes its heads' attention and a partial output projection
(W_o rows for its heads); the host sums the 4 partials per batch + b_o.

All operands fp16 (fp32 PSUM accumulation).  Per-core structure, emitted as
one software-pipelined stream so the PE never idles long enough to re-throttle:

  proj(c):  Q/K projected with stationary W slices -> psum arrives [d, q];
            RoPE as A=ps*cos, B=ps*sin (DVE) then rot = I@A + Msw@B on the
            PE (Msw is a signed even/odd swap matrix), evacuated fp16.
            V projected naturally (stationary X^T tiles), stored [k,h,65]
            with a ones column for the softmax denominator.
  attn(c):  per (pair, 2-k-tile group): transposed scores S^T[k,q] for both
            heads of the pair issued back-to-back as 64-row matmuls at
            tile_position (0,0)/(64,0) so they run concurrently; ScalarE
            exp (scale=1/8, 2x perf mode via fp16 out); diagonal blocks
            masked multiplicatively with a precomputed 0/1 triangle (DVE);
            PV with stationary V_ext=[V | 1s] accumulates ctx^T with the
            denominator in row 64.  Softmax division via ScalarE
            rec = exp(-ln(den)) (one table set has both Ln and Exp),
            GPSIMD partition-broadcast, one DVE multiply into ctxT fp16.
  out(c):   partial out = ctx @ W_o, fp16 to DRAM.

proj(c+1) is interleaved into attn(c) (one step per k-tile group) to keep
the tensor engine dense; out-proj of chunk c fills the gap before attn(c+1).

Softmax skips the max-subtraction: scores here are bounded (|s| < ~3) and
exp is computed in fp32, so softmax shift-invariance makes it exact enough.
"""

import sys

for _p in ("/opt/trn_rl_repo",):
    if _p not in sys.path:
        sys.path.insert(0, _p)

from contextlib import ExitStack

import numpy as np

import concourse.bass as bass
import concourse.mybir as mybir
import concourse.tile as tile
from concourse import bacc

F16 = np.float16

B = 2
S = 2048
D = 1024
H = 16
DK = 64
HPC = 4  # heads per core
DC = HPC * DK  # 256 model dims per core
N_CORES = 8
SCALE = 1.0 / np.sqrt(DK)
QT = S // 128  # 16 q tiles
KTILES = S // 128
MKT = 8  # model-dim k-tiles (1024/128)
NCH = 4  # 512-wide q chunks

_PROG_CACHE = {}


def _build_program():
    nc = bacc.Bacc("TRN2", target_bir_lowering=False, debug=False)
    f32 = mybir.dt.float32
    f16 = mybir.dt.float16

    # ---- DRAM I/O ----
    xqT = nc.dram_tensor("xqT", [128, NCH, MKT, 512], f16, kind="ExternalInput")
    xkT = nc.dram_tensor("xkT", [128, NCH, MKT, 512], f16, kind="ExternalInput")
    xvT = nc.dram_tensor("xvT", [128, NCH, MKT, 512], f16, kind="ExternalInput")
    wqk = nc.dram_tensor("wqk", [128, MKT, 2 * DC], f16, kind="ExternalInput")
    wv = nc.dram_tensor("wv", [128, MKT, DC], f16, kind="ExternalInput")
    wo = nc.dram_tensor("wo", [128, 2, D], f16, kind="ExternalInput")
    cosd = nc.dram_tensor("cosd", [128, S], f16, kind="ExternalInput")
    sind = nc.dram_tensor("sind", [128, S], f16, kind="ExternalInput")
    mswd = nc.dram_tensor("mswd", [128, 128], f16, kind="ExternalInput")
    idend = nc.dram_tensor("idend", [128, 128], f16, kind="ExternalInput")
    trid = nc.dram_tensor("trid", [128, 128], f16, kind="ExternalInput")
    out = nc.dram_tensor("out", [S, D], f16, kind="ExternalOutput")

    with tile.TileContext(nc) as tc, ExitStack() as top:
        persist = top.enter_context(tc.tile_pool(name="persist", bufs=1))
        px = top.enter_context(tc.tile_pool(name="px", bufs=2))
        pab = top.enter_context(tc.tile_pool(name="pab", bufs=2))
        pexp = top.enter_context(tc.tile_pool(name="pexp", bufs=2))
        pden = top.enter_context(tc.tile_pool(name="pden", bufs=2))
        pbc = top.enter_context(tc.tile_pool(name="pbc", bufs=2))
        posb = top.enter_context(tc.tile_pool(name="posb", bufs=4))
        pwork = top.enter_context(tc.tile_pool(name="pwork", bufs=2, space="PSUM"))
        psc = top.enter_context(tc.tile_pool(name="psc", bufs=1, space="PSUM"))
        pctx = top.enter_context(tc.tile_pool(name="pctx", bufs=1, space="PSUM"))

        # persistent SBUF tensors
        wv_sb = persist.tile([128, MKT, DC], f16, tag="wv")
        wqk_sb = persist.tile([128, MKT, 2 * DC], f16, tag="wqk")
        wo_sb = persist.tile([128, 2, D], f16, tag="wo")
        cos_sb = persist.tile([128, S], f16, tag="cos")
        sin_sb = persist.tile([128, S], f16, tag="sin")
        msw_sb = persist.tile([128, 128], f16, tag="msw")
        iden_sb = persist.tile([128, 128], f16, tag="iden")
        tri_sb = persist.tile([128, 128], f16, tag="tri")
        qtT = persist.tile([128, 2, QT, 128], f16, tag="qtT")
        ktT = persist.tile([128, 2, QT, 128], f16, tag="ktT")
        v_sb = persist.tile([128, KTILES, HPC, 65], f16, tag="v")
        ctxT = persist.tile([128, 2, QT, 128], f16, tag="ctxT")
        dum = persist.tile([1, 16], f32, tag="dum")
        warm = persist.tile([128, 512], f16, tag="warm")

        def load_chunk(c, eng=None):
            t = {}
            for key, dram in (("v", xvT), ("q", xqT), ("k", xkT)):
                xt = px.tile([128, MKT, 512], f16, tag=f"x{key}", name=f"x{key}_t")
                (eng or nc.sync).dma_start(xt[:], dram[:, c])
                t[key] = xt
            return t

        def alloc_chunk():
            return {
                key: px.tile([128, MKT, 512], f16, tag=f"x{key}", name=f"x{key}_t")
                for key in ("v", "q", "k")
            }

        # initial loads spread across the three DMA-capable engine queues
        # (sync/scalar/gpsimd); a single queue ramps slowly and would gate
        # the first projections.  Priority: wv+xv0 (V proj), wqk+xq0+trig
        # (Q/K proj + RoPE), then xk0, chunk 1, tri, wo.
        xts = {0: alloc_chunk(), 1: alloc_chunk()}
        nc.gpsimd.memset(warm[:], 0.0)
        nc.gpsimd.dma_start(wv_sb[:], wv[:])
        nc.gpsimd.dma_start(xts[0]["v"][:, 4:8, :], xvT[:, 0, 4:8, :])
        nc.gpsimd.dma_start(msw_sb[:], mswd[:])
        nc.gpsimd.dma_start(iden_sb[:], idend[:])
        nc.gpsimd.memset(v_sb[:, :, :, 64:65], 1.0)
        nc.gpsimd.memset(dum[:], 1.0)
        nc.gpsimd.dma_start(xts[1]["v"][:], xvT[:, 1])
        nc.gpsimd.dma_start(xts[1]["q"][:], xqT[:, 1])
        nc.gpsimd.dma_start(xts[1]["k"][:], xkT[:, 1])
        nc.sync.dma_start(xts[0]["v"][:, 0:4, :], xvT[:, 0, 0:4, :])
        nc.sync.dma_start(xts[0]["q"][:], xqT[:, 0])
        nc.sync.dma_start(xts[0]["k"][:], xkT[:, 0])
        # preload the combined Ln+Exp table set during the initial DMAs; the
        # explicit load makes the insert_act_table_loads fixpoint see both
        # functions resident so it does not thrash between the Exp-only and
        # Ln-only sets on every softmax denominator.
        nc.scalar.add_instruction(
            mybir.InstLoadActFuncSet(
                name=nc.get_next_instruction_name(), act_func_set_id=6
            )
        )
        nc.scalar.dma_start(wqk_sb[:], wqk[:])
        nc.scalar.dma_start(cos_sb[:], cosd[:])
        nc.scalar.dma_start(sin_sb[:], sind[:])
        nc.scalar.activation(dum[:], dum[:], mybir.ActivationFunctionType.Exp)
        nc.scalar.activation(dum[:], dum[:], mybir.ActivationFunctionType.Ln)
        nc.scalar.dma_start(tri_sb[:], trid[:])
        nc.scalar.dma_start(wo_sb[:], wo[:])
        # dummy matmul stream over the initial DMA wait: sustained PE
        # activity flips the HAM clock gate to 8/8 before the real work
        wm_ps = pwork.tile([128, 512], f32, tag="w", name="wm_ps")
        for _ in range(24):
            nc.tensor.matmul(
                wm_ps[:], lhsT=warm[:, 0:128], rhs=warm[:], start=True, stop=True
            )

        def emit_rot(a_t, b_t, dstT, pair, c):
            rot = pwork.tile([128, 512], f32, tag="w", name="rot_ps")
            nc.tensor.matmul(rot[:], lhsT=iden_sb[:], rhs=a_t[:], start=True, stop=False)
            nc.tensor.matmul(rot[:], lhsT=msw_sb[:], rhs=b_t[:], start=False, stop=True)
            nc.vector.tensor_copy(
                dstT[:, pair, 4 * c : 4 * c + 4, :],
                rot[:].rearrange("p (t q) -> p t q", t=4),
            )

        def proj_stream(c, xt):
            csl = slice(c * 512, (c + 1) * 512)
            # V projection: natural layout, stationary X^T tiles
            for qq in range(4):
                v_ps = pwork.tile([128, 512], f32, tag="w", name="v_ps")
                for kt in range(MKT):
                    nc.tensor.matmul(
                        v_ps[:, 0:DC],
                        lhsT=xt["v"][:, kt, qq * 128 : (qq + 1) * 128],
                        rhs=wv_sb[:, kt, :],
                        start=(kt == 0),
                        stop=(kt == MKT - 1),
                    )
                nc.vector.tensor_copy(
                    v_sb[:, 4 * c + qq, :, 0:64],
                    v_ps[:, 0:DC].rearrange("p (h d) -> p h d", h=HPC),
                )
                yield
            # Q/K projections with RoPE (rot matmuls lag one iteration)
            pend = None
            for name_off, x_key, dstT in ((0, "q", qtT), (2 * 128, "k", ktT)):
                for pair in range(2):
                    wsl = slice(name_off + pair * 128, name_off + (pair + 1) * 128)
                    ps = pwork.tile([128, 512], f32, tag="w", name="pj_ps")
                    for kt in range(MKT):
                        nc.tensor.matmul(
                            ps[:],
                            lhsT=wqk_sb[:, kt, wsl],
                            rhs=xt[x_key][:, kt, :],
                            start=(kt == 0),
                            stop=(kt == MKT - 1),
                        )
                    a_t = pab.tile([128, 512], f16, tag="a", name="a_t")
                    nc.vector.tensor_mul(a_t[:], ps[:], cos_sb[:, csl])
                    b_t = pab.tile([128, 512], f16, tag="b", name="b_t")
                    nc.vector.tensor_mul(b_t[:], ps[:], sin_sb[:, csl])
                    if pend is not None:
                        emit_rot(*pend)
                    pend = (a_t, b_t, dstT, pair, c)
                    yield
            emit_rot(*pend)
            yield

        def drain(gens, n=1):
            for _ in range(n):
                for g in gens:
                    if g is None:
                        continue
                    try:
                        next(g)
                        break
                    except StopIteration:
                        continue

        def attn_pair(c, pair, gens):
            nk = 4 * (c + 1)
            qsl = slice(4 * c, 4 * c + 4)
            ctx2 = pctx.tile([65, 2, 512], f32, tag="ctx", name="ctx2")
            prev = None  # (kts, exps)
            for gp in range(nk // 2):
                kts = (2 * gp, 2 * gp + 1)
                diag = gp >= 2 * c
                # per-head score tiles: head A frees after exp_A, so the next
                # group's A-scores overlap exp_B (PE-bound steady state)
                scs = [
                    psc.tile([128, 2, 512], f32, tag=t, name=f"sc{t}")
                    for t in ("A", "B")
                ]
                exps = [
                    pexp.tile([128, 2, 512], f16, tag=t, name=f"e{t}")
                    for t in ("eA", "eB")
                ]
                # scores: full width (so exp reads fully-written tiles);
                # A/B adjacent for row-group concurrency
                for j, kt in enumerate(kts):
                    for hh in range(2):
                        d0 = hh * 64
                        nc.tensor.matmul(
                            scs[hh][:, j, :],
                            lhsT=ktT[d0 : d0 + 64, pair, kt, :],
                            rhs=qtT[d0 : d0 + 64, pair, qsl, :],
                            start=True,
                            stop=True,
                        )
                # PV of the previous group (software pipeline, lag 1)
                if prev is not None:
                    emit_pv(c, pair, ctx2, *prev, nk)
                for hh in range(2):
                    # exp full width (PV trims skip the sub-diagonal region)
                    nc.scalar.activation(
                        exps[hh][:],
                        scs[hh][:],
                        mybir.ActivationFunctionType.Exp,
                        scale=float(SCALE),
                    )
                if diag:
                    # multiplicative 0/1 triangle on the diagonal blocks
                    for hh in range(2):
                        for j, kt in enumerate(kts):
                            qo = kt - 4 * c
                            nc.vector.tensor_mul(
                                exps[hh][:, j, qo * 128 : (qo + 1) * 128],
                                exps[hh][:, j, qo * 128 : (qo + 1) * 128],
                                tri_sb[:],
                            )
                drain(gens)
                prev = (kts, exps)
            emit_pv(c, pair, ctx2, *prev, nk)
            # denominators: rec = exp(-ln(den)); row 64 of the ctx psum
            lnt = pden.tile([1, 2, 512], f32, tag="ln", name="lnt")
            nc.scalar.activation(
                lnt[0:1, :, :], ctx2[64:65, :, :], mybir.ActivationFunctionType.Ln
            )
            rec = pden.tile([1, 2, 512], f32, tag="rec", name="rec")
            nc.scalar.activation(
                rec[:], lnt[:], mybir.ActivationFunctionType.Exp, scale=-1.0
            )
            for hh in range(2):
                bc = pbc.tile([64, 512], f32, tag=f"bc{hh}", name="bc")
                nc.gpsimd.partition_broadcast(bc[:], rec[0:1, hh, :])
                d0 = hh * 64
                nc.vector.tensor_mul(
                    ctxT[d0 : d0 + 64, pair, qsl, :],
                    ctx2[0:64, hh, :].rearrange("p (t q) -> p t q", t=4),
                    bc[:].rearrange("p (t q) -> p t q", t=4),
                )
            # queue PE filler over the serial den/normalize chain so the
            # tensor engine does not idle into a HAM re-throttle
            drain(gens, 2)

        def emit_pv(c, pair, ctx2, kts, exps, nk):
            for j, kt in enumerate(kts):
                qo = max(0, kt - 4 * c)
                for hh in range(2):
                    nc.tensor.matmul(
                        ctx2[:, hh, qo * 128 :],
                        lhsT=v_sb[:, kt, 2 * pair + hh, :],
                        rhs=exps[hh][:, j, qo * 128 :],
                        start=(kt == 0),
                        stop=(kt == nk - 1),
                    )

        def outproj_stream(c):
            for qt in range(4 * c, 4 * c + 4):
                for ec in range(2):
                    ops = pwork.tile([128, 512], f32, tag="w", name="o_ps")
                    for pair in range(2):
                        nc.tensor.matmul(
                            ops[:],
                            lhsT=ctxT[:, pair, qt, :],
                            rhs=wo_sb[:, pair, ec * 512 : (ec + 1) * 512],
                            start=(pair == 0),
                            stop=(pair == 1),
                        )
                    osb = posb.tile([128, 512], f16, tag="osb", name="osb")
                    nc.vector.tensor_copy(osb[:], ops[:])
                    nc.sync.dma_start(
                        out[qt * 128 : (qt + 1) * 128, ec * 512 : (ec + 1) * 512],
                        osb[:],
                    )
                    yield

        # ---------------- main emission ----------------
        for _ in proj_stream(0, xts[0]):
            pass
        ogen = None
        for c in range(NCH):
            if c + 2 < NCH:
                xts[c + 2] = load_chunk(c + 2)
            pgen = proj_stream(c + 1, xts[c + 1]) if c + 1 < NCH else None
            gens = (pgen, ogen)
            attn_pair(c, 0, gens)
            attn_pair(c, 1, gens)
            drain(gens, 100)
            ogen = outproj_stream(c)
        drain((ogen,), 100)

    if not nc.is_finalized():
        nc.finalize()
    return nc


def _prep_core_inputs(inputs):
    """Build the 8 per-core input maps (host-side shard + transpose + cast)."""
    qx = np.asarray(inputs["q_input"], np.float32)
    kx = np.asarray(inputs["k_input"], np.float32)
    vx = np.asarray(inputs["v_input"], np.float32)
    W_q = np.asarray(inputs["W_q"], np.float32)
    W_k = np.asarray(inputs["W_k"], np.float32)
    W_v = np.asarray(inputs["W_v"], np.float32)
    W_o = np.asarray(inputs["W_o"], np.float32)
    for bn in ("b_q", "b_k", "b_v"):
        assert not np.any(np.asarray(inputs[bn])), f"nonzero {bn} unsupported"

    # RoPE column permutation: within each head, evens then odds
    perm = np.concatenate(
        [
            h * DK + np.concatenate([np.arange(0, DK, 2), np.arange(1, DK, 2)])
            for h in range(H)
        ]
    )
    W_q_p = W_q[:, perm]
    W_k_p = W_k[:, perm]

    # replicated trig tables for transposed-layout RoPE: [p, s], p%32 = pair idx
    theta = 10000.0 ** (-2.0 * np.arange(32, dtype=np.float64) / DK)
    pos = np.arange(S, dtype=np.float64)
    angT = theta[:, None] * pos[None, :]  # [32, S]
    cosT = np.tile(np.cos(angT), (4, 1)).astype(F16)  # [128, S]
    sinT = np.tile(np.sin(angT), (4, 1)).astype(F16)

    # signed even/odd swap matrix: rot = I@(ps*cos) + Msw@(ps*sin)
    msw = np.zeros((128, 128), np.float32)
    for h in range(2):
        for i in range(32):
            msw[h * 64 + 32 + i, h * 64 + i] = -1.0  # re -= o*sin
            msw[h * 64 + i, h * 64 + 32 + i] = 1.0  # ro += e*sin
    iden = np.eye(128, dtype=np.float32)
    tri = (np.arange(128)[None, :] >= np.arange(128)[:, None]).astype(np.float32)

    def shard_xT(x_b):  # [S, D] -> [128, NCH, MKT, 512] f16
        t = x_b.T.astype(F16).reshape(MKT, 128, NCH, 512)
        return t.transpose(1, 2, 0, 3).copy()

    in_maps = []
    for c in range(N_CORES):
        b = c // 4
        g = c % 4
        cols = slice(g * DC, (g + 1) * DC)
        m = {
            "xqT": shard_xT(qx[b]),
            "xkT": shard_xT(kx[b]),
            "xvT": shard_xT(vx[b]),
            "wqk": np.concatenate([W_q_p[:, cols], W_k_p[:, cols]], axis=1)
            .astype(F16)
            .reshape(MKT, 128, 2 * DC)
            .transpose(1, 0, 2)
            .copy(),
            "wv": W_v[:, cols].astype(F16).reshape(MKT, 128, DC).transpose(1, 0, 2).copy(),
            "wo": W_o[cols, :].astype(F16).reshape(2, 128, D).transpose(1, 0, 2).copy(),
            "cosd": cosT,
            "sind": sinT,
            "mswd": msw.astype(F16),
            "idend": iden.astype(F16),
            "trid": tri.astype(F16),
        }
        in_maps.append(m)
    return in_maps


def _check_causal(mask):
    mask = np.asarray(mask)
    jj = np.arange(S)
    assert np.array_equal(mask != 0, jj[None, :] <= jj[:, None]), (
        "only the causal mask is supported"
    )


def _run(inputs, trace=False, tmpdir=None, sim=False, sim_cores=(0,)):
    from concourse.bass_utils import run_bass_kernel_spmd

    _check_causal(inputs["mask"])
    in_maps = _prep_core_inputs(inputs)

    if "prog" not in _PROG_CACHE:
        _PROG_CACHE["prog"] = _build_program()
    nc = _PROG_CACHE["prog"]

    b_o = np.asarray(inputs["b_o"], np.float32)

    if sim:
        from concourse.bass_interp import CoreSim

        partials = {}
        for c in sim_cores:
            simr = CoreSim(nc)
            for name, val in in_maps[c].items():
                simr.tensor(name)[:] = val
            simr.simulate()
            partials[c] = np.array(simr.tensor("out"))
        return partials, None

    res = run_bass_kernel_spmd(
        nc, in_maps, list(range(N_CORES)), trace=trace, tmpdir=tmpdir
    )
    outs = [res.results[c]["out"].astype(np.float32) for c in range(N_CORES)]
    full = np.zeros((B, S, D), np.float32)
    for b in range(B):
        full[b] = outs[4 * b] + outs[4 * b + 1] + outs[4 * b + 2] + outs[4 * b + 3]
        full[b] += b_o[None, :]
    return full, res


def kernel(**inputs) -> np.ndarray:
    out, _ = _run(inputs, trace=False)
    return out


# revision 19
# speedup vs baseline: 1.0058x; 1.0058x over previous
"""Multi-head attention (RoPE + causal softmax) Bass kernel for 8 TRN2 cores.

Problem: B=2, S=2048, D=1024, H=16 heads, d_k=64.
Sharding: data-parallel over batch (2) x tensor-parallel over heads (4 groups
of 4 heads).  Core c handles batch c//4, heads [4*(c%4), 4*(c%4)+4).
Each core computes its heads' attention and a partial output projection
(W_o rows for its heads); the host sums the 4 partials per batch + b_o.

All operands fp16 (fp32 PSUM accumulation).  Per-core structure, emitted as
one software-pipelined stream so the PE never idles long enough to re-throttle:

  proj(c):  Q/K projected with stationary W slices -> psum arrives [d, q];
            RoPE as A=ps*cos, B=ps*sin (DVE) then rot = I@A + Msw@B on the
            PE (Msw is a signed even/odd swap matrix), evacuated fp16.
            V projected naturally (stationary X^T tiles), stored [k,h,65]
            with a ones column for the softmax denominator.
  attn(c):  per (pair, 2-k-tile group): transposed scores S^T[k,q] for both
            heads of the pair issued back-to-back as 64-row matmuls at
            tile_position (0,0)/(64,0) so they run concurrently; ScalarE
            exp (scale=1/8, 2x perf mode via fp16 out); diagonal blocks
            masked multiplicatively with a precomputed 0/1 triangle (DVE);
            PV with stationary V_ext=[V | 1s] accumulates ctx^T with the
            denominator in row 64.  Softmax division via ScalarE
            rec = exp(-ln(den)) (one table set has both Ln and Exp),
            GPSIMD partition-broadcast, one DVE multiply into ctxT fp16.
  out(c):   partial out = ctx @ W_o, fp16 to DRAM.

proj(c+1) is interleaved into attn(c) (one step per k-tile group) to keep
the tensor engine dense; out-proj of chunk c fills the gap before attn(c+1).

Softmax skips the max-subtraction: scores here are bounded (|s| < ~3) and
exp is computed in fp32, so softmax shift-invariance makes it exact enough.
"""

import sys

for _p in ("/opt/trn_rl_repo",):
    if _p not in sys.path:
        sys.path.insert(0, _p)

from contextlib import ExitStack

import numpy as np

import concourse.bass as bass
import concourse.mybir as mybir
import concourse.tile as tile
from concourse import bacc

F16 = np.float16

B = 2
S = 2048
D = 1024
H = 16
DK = 64
HPC = 4  # heads per core
DC = HPC * DK  # 256 model dims per core
N_CORES = 8
SCALE = 1.0 / np.sqrt(DK)
QT = S // 128  # 16 q tiles
KTILES = S // 128
MKT = 8  # model-dim k-tiles (1024/128)
NCH = 4  # 512-wide q chunks

_PROG_CACHE = {}


def _build_program():
    nc = bacc.Bacc("TRN2", target_bir_lowering=False, debug=False)
    f32 = mybir.dt.float32
    f16 = mybir.dt.float16

    # ---- DRAM I/O ----
    xqT = nc.dram_tensor("xqT", [128, NCH, MKT, 512], f16, kind="ExternalInput")
    xkT = nc.dram_tensor("xkT", [128, NCH, MKT, 512], f16, kind="ExternalInput")
    xvT = nc.dram_tensor("xvT", [128, NCH, MKT, 512], f16, kind="ExternalInput")
    wqk = nc.dram_tensor("wqk", [128, MKT, 2 * DC], f16, kind="ExternalInput")
    wv = nc.dram_tensor("wv", [128, MKT, DC], f16, kind="ExternalInput")
    wo = nc.dram_tensor("wo", [128, 2, D], f16, kind="ExternalInput")
    cosd = nc.dram_tensor("cosd", [128, S], f16, kind="ExternalInput")
    sind = nc.dram_tensor("sind", [128, S], f16, kind="ExternalInput")
    mswd = nc.dram_tensor("mswd", [128, 128], f16, kind="ExternalInput")
    idend = nc.dram_tensor("idend", [128, 128], f16, kind="ExternalInput")
    trid = nc.dram_tensor("trid", [128, 128], f16, kind="ExternalInput")
    out = nc.dram_tensor("out", [S, D], f16, kind="ExternalOutput")

    with tile.TileContext(nc) as tc, ExitStack() as top:
        persist = top.enter_context(tc.tile_pool(name="persist", bufs=1))
        px = top.enter_context(tc.tile_pool(name="px", bufs=2))
        pab = top.enter_context(tc.tile_pool(name="pab", bufs=2))
        pexp = top.enter_context(tc.tile_pool(name="pexp", bufs=2))
        pden = top.enter_context(tc.tile_pool(name="pden", bufs=2))
        pbc = top.enter_context(tc.tile_pool(name="pbc", bufs=2))
        posb = top.enter_context(tc.tile_pool(name="posb", bufs=4))
        pwork = top.enter_context(tc.tile_pool(name="pwork", bufs=2, space="PSUM"))
        psc = top.enter_context(tc.tile_pool(name="psc", bufs=1, space="PSUM"))
        pctx = top.enter_context(tc.tile_pool(name="pctx", bufs=1, space="PSUM"))

        # persistent SBUF tensors
        wv_sb = persist.tile([128, MKT, DC], f16, tag="wv")
        wqk_sb = persist.tile([128, MKT, 2 * DC], f16, tag="wqk")
        wo_sb = persist.tile([128, 2, D], f16, tag="wo")
        cos_sb = persist.tile([128, S], f16, tag="cos")
        sin_sb = persist.tile([128, S], f16, tag="sin")
        msw_sb = persist.tile([128, 128], f16, tag="msw")
        iden_sb = persist.tile([128, 128], f16, tag="iden")
        tri_sb = persist.tile([128, 128], f16, tag="tri")
        qtT = persist.tile([128, 2, QT, 128], f16, tag="qtT")
        ktT = persist.tile([128, 2, QT, 128], f16, tag="ktT")
        v_sb = persist.tile([128, KTILES, HPC, 65], f16, tag="v")
        ctxT = persist.tile([128, 2, QT, 128], f16, tag="ctxT")
        dum = persist.tile([1, 16], f32, tag="dum")
        warm = persist.tile([128, 512], f16, tag="warm")

        def load_chunk(c, eng=None):
            t = {}
            for key, dram in (("v", xvT), ("q", xqT), ("k", xkT)):
                xt = px.tile([128, MKT, 512], f16, tag=f"x{key}", name=f"x{key}_t")
                (eng or nc.sync).dma_start(xt[:], dram[:, c])
                t[key] = xt
            return t

        def alloc_chunk():
            return {
                key: px.tile([128, MKT, 512], f16, tag=f"x{key}", name=f"x{key}_t")
                for key in ("v", "q", "k")
            }

        # initial loads spread across the three DMA-capable engine queues
        # (sync/scalar/gpsimd), ordered by when each tensor is first needed;
        # a single queue sustains only ~100GB/s early and would serialize
        # the whole projection front.
        xts = {0: alloc_chunk(), 1: alloc_chunk()}
        nc.vector.memset(warm[:], 0.0)
        nc.vector.memset(dum[:], 1.0)
        nc.vector.memset(v_sb[:, :, :, 64:65], 1.0)
        x0, x1 = xts[0], xts[1]
        nc.sync.dma_start(x0["v"][:, 0:4, :], xvT[:, 0, 0:4, :])
        nc.sync.dma_start(x0["v"][:, 4:8, :], xvT[:, 0, 4:8, :])
        nc.sync.dma_start(x0["q"][:, 0:4, :], xqT[:, 0, 0:4, :])
        nc.sync.dma_start(x0["k"][:, 0:4, :], xkT[:, 0, 0:4, :])
        nc.sync.dma_start(x1["v"][:, 0:4, :], xvT[:, 1, 0:4, :])
        nc.sync.dma_start(x1["v"][:, 4:8, :], xvT[:, 1, 4:8, :])
        # preload the combined Ln+Exp table set during the initial DMAs; the
        # explicit load makes the insert_act_table_loads fixpoint see both
        # functions resident so it does not thrash between the Exp-only and
        # Ln-only sets on every softmax denominator.
        nc.scalar.add_instruction(
            mybir.InstLoadActFuncSet(
                name=nc.get_next_instruction_name(), act_func_set_id=6
            )
        )
        nc.scalar.dma_start(wv_sb[:], wv[:])
        nc.scalar.dma_start(wqk_sb[:, :, 0:DC], wqk[:, :, 0:DC])
        nc.scalar.dma_start(cos_sb[:], cosd[:])
        nc.scalar.dma_start(msw_sb[:], mswd[:])
        nc.scalar.dma_start(iden_sb[:], idend[:])
        nc.scalar.activation(dum[:], dum[:], mybir.ActivationFunctionType.Exp)
        nc.scalar.activation(dum[:], dum[:], mybir.ActivationFunctionType.Ln)
        nc.scalar.dma_start(tri_sb[:], trid[:])
        nc.scalar.dma_start(x1["q"][:, 0:4, :], xqT[:, 1, 0:4, :])
        nc.scalar.dma_start(wo_sb[:], wo[:])
        nc.gpsimd.dma_start(wqk_sb[:, :, DC : 2 * DC], wqk[:, :, DC : 2 * DC])
        nc.gpsimd.dma_start(sin_sb[:], sind[:])
        nc.gpsimd.dma_start(x0["q"][:, 4:8, :], xqT[:, 0, 4:8, :])
        nc.gpsimd.dma_start(x0["k"][:, 4:8, :], xkT[:, 0, 4:8, :])
        nc.gpsimd.dma_start(x1["q"][:, 4:8, :], xqT[:, 1, 4:8, :])
        nc.gpsimd.dma_start(x1["k"][:, 0:4, :], xkT[:, 1, 0:4, :])
        nc.gpsimd.dma_start(x1["k"][:, 4:8, :], xkT[:, 1, 4:8, :])
        # dummy matmul stream over the initial DMA wait: sustained PE
        # activity flips the HAM clock gate to 8/8 before the real work
        wm_ps = pwork.tile([128, 512], f32, tag="w", name="wm_ps")
        for _ in range(24):
            nc.tensor.matmul(
                wm_ps[:], lhsT=warm[:, 0:128], rhs=warm[:], start=True, stop=True
            )

        def emit_rot(a_t, b_t, dstT, pair, c):
            rot = pwork.tile([128, 512], f32, tag="w", name="rot_ps")
            nc.tensor.matmul(rot[:], lhsT=iden_sb[:], rhs=a_t[:], start=True, stop=False)
            nc.tensor.matmul(rot[:], lhsT=msw_sb[:], rhs=b_t[:], start=False, stop=True)
            nc.vector.tensor_copy(
                dstT[:, pair, 4 * c : 4 * c + 4, :],
                rot[:].rearrange("p (t q) -> p t q", t=4),
            )

        def proj_stream(c, xt):
            csl = slice(c * 512, (c + 1) * 512)
            # V projection: natural layout, stationary X^T tiles
            for qq in range(4):
                v_ps = pwork.tile([128, 512], f32, tag="w", name="v_ps")
                for kt in range(MKT):
                    nc.tensor.matmul(
                        v_ps[:, 0:DC],
                        lhsT=xt["v"][:, kt, qq * 128 : (qq + 1) * 128],
                        rhs=wv_sb[:, kt, :],
                        start=(kt == 0),
                        stop=(kt == MKT - 1),
                    )
                nc.vector.tensor_copy(
                    v_sb[:, 4 * c + qq, :, 0:64],
                    v_ps[:, 0:DC].rearrange("p (h d) -> p h d", h=HPC),
                )
                yield
            # Q/K projections with RoPE (rot matmuls lag one iteration)
            pend = None
            for name_off, x_key, dstT in ((0, "q", qtT), (2 * 128, "k", ktT)):
                for pair in range(2):
                    wsl = slice(name_off + pair * 128, name_off + (pair + 1) * 128)
                    ps = pwork.tile([128, 512], f32, tag="w", name="pj_ps")
                    for kt in range(MKT):
                        nc.tensor.matmul(
                            ps[:],
                            lhsT=wqk_sb[:, kt, wsl],
                            rhs=xt[x_key][:, kt, :],
                            start=(kt == 0),
                            stop=(kt == MKT - 1),
                        )
                    a_t = pab.tile([128, 512], f16, tag="a", name="a_t")
                    nc.vector.tensor_mul(a_t[:], ps[:], cos_sb[:, csl])
                    b_t = pab.tile([128, 512], f16, tag="b", name="b_t")
                    nc.vector.tensor_mul(b_t[:], ps[:], sin_sb[:, csl])
                    if pend is not None:
                        emit_rot(*pend)
                    pend = (a_t, b_t, dstT, pair, c)
                    yield
            emit_rot(*pend)
            yield

        def drain(gens, n=1):
            for _ in range(n):
                for g in gens:
                    if g is None:
                        continue
                    try:
                        next(g)
                        break
                    except StopIteration:
                        continue

        def attn_pair(c, pair, gens):
            nk = 4 * (c + 1)
            qsl = slice(4 * c, 4 * c + 4)
            ctx2 = pctx.tile([65, 2, 512], f32, tag="ctx", name="ctx2")
            prev = None  # (kts, exps)
            for gp in range(nk // 2):
                kts = (2 * gp, 2 * gp + 1)
                diag = gp >= 2 * c
                # per-head score tiles: head A frees after exp_A, so the next
                # group's A-scores overlap exp_B (PE-bound steady state)
                scs = [
                    psc.tile([128, 2, 512], f32, tag=t, name=f"sc{t}")
                    for t in ("A", "B")
                ]
                exps = [
                    pexp.tile([128, 2, 512], f16, tag=t, name=f"e{t}")
                    for t in ("eA", "eB")
                ]
                # scores: full width (so exp reads fully-written tiles);
                # A/B adjacent for row-group concurrency
                for j, kt in enumerate(kts):
                    for hh in range(2):
                        d0 = hh * 64
                        nc.tensor.matmul(
                            scs[hh][:, j, :],
                            lhsT=ktT[d0 : d0 + 64, pair, kt, :],
                            rhs=qtT[d0 : d0 + 64, pair, qsl, :],
                            start=True,
                            stop=True,
                        )
                # PV of the previous group (software pipeline, lag 1)
                if prev is not None:
                    emit_pv(c, pair, ctx2, *prev, nk)
                for hh in range(2):
                    # exp full width (PV trims skip the sub-diagonal region)
                    nc.scalar.activation(
                        exps[hh][:],
                        scs[hh][:],
                        mybir.ActivationFunctionType.Exp,
                        scale=float(SCALE),
                    )
                if diag:
                    # multiplicative 0/1 triangle on the diagonal blocks
                    for hh in range(2):
                        for j, kt in enumerate(kts):
                            qo = kt - 4 * c
                            nc.vector.tensor_mul(
                                exps[hh][:, j, qo * 128 : (qo + 1) * 128],
                                exps[hh][:, j, qo * 128 : (qo + 1) * 128],
                                tri_sb[:],
                            )
                drain(gens)
                prev = (kts, exps)
            emit_pv(c, pair, ctx2, *prev, nk)
            # denominators: rec = exp(-ln(den)); row 64 of the ctx psum
            lnt = pden.tile([1, 2, 512], f32, tag="ln", name="lnt")
            nc.scalar.activation(
                lnt[0:1, :, :], ctx2[64:65, :, :], mybir.ActivationFunctionType.Ln
            )
            rec = pden.tile([1, 2, 512], f32, tag="rec", name="rec")
            nc.scalar.activation(
                rec[:], lnt[:], mybir.ActivationFunctionType.Exp, scale=-1.0
            )
            for hh in range(2):
                bc = pbc.tile([64, 512], f32, tag=f"bc{hh}", name="bc")
                nc.gpsimd.partition_broadcast(bc[:], rec[0:1, hh, :])
                d0 = hh * 64
                nc.vector.tensor_mul(
                    ctxT[d0 : d0 + 64, pair, qsl, :],
                    ctx2[0:64, hh, :].rearrange("p (t q) -> p t q", t=4),
                    bc[:].rearrange("p (t q) -> p t q", t=4),
                )
            # queue PE filler over the serial den/normalize chain so the
            # tensor engine does not idle into a HAM re-throttle
            drain(gens, 2)

        def emit_pv(c, pair, ctx2, kts, exps, nk):
            for j, kt in enumerate(kts):
                qo = max(0, kt - 4 * c)
                for hh in range(2):
                    nc.tensor.matmul(
                        ctx2[:, hh, qo * 128 :],
                        lhsT=v_sb[:, kt, 2 * pair + hh, :],
                        rhs=exps[hh][:, j, qo * 128 :],
                        start=(kt == 0),
                        stop=(kt == nk - 1),
                    )

        oq = [nc.sync, nc.scalar, nc.gpsimd]

        def outproj_stream(c):
            for qt in range(4 * c, 4 * c + 4):
                for ec in range(2):
                    ops = pwork.tile([128, 512], f32, tag="w", name="o_ps")
                    for pair in range(2):
                        nc.tensor.matmul(
                            ops[:],
                            lhsT=ctxT[:, pair, qt, :],
                            rhs=wo_sb[:, pair, ec * 512 : (ec + 1) * 512],
                            start=(pair == 0),
                            stop=(pair == 1),
                        )
                    osb = posb.tile([128, 512], f16, tag="osb", name="osb")
                    nc.vector.tensor_copy(osb[:], ops[:])
                    oq[(2 * qt + ec) % 3].dma_start(
                        out[qt * 128 : (qt + 1) * 128, ec * 512 : (ec + 1) * 512],
                        osb[:],
                    )
                    yield

        # ---------------- main emission ----------------
        for _ in proj_stream(0, xts[0]):
            pass
        ogen = None
        for c in range(NCH):
            if c + 2 < NCH:
                xts[c + 2] = load_chunk(c + 2)
            pgen = proj_stream(c + 1, xts[c + 1]) if c + 1 < NCH else None
            gens = (pgen, ogen)
            attn_pair(c, 0, gens)
            attn_pair(c, 1, gens)
            # previous chunk's out-proj remainder is ready work; the proj
            # remainder for c+1 may still be waiting on its x DMA
            drain((ogen,), 100)
            if c == 0:
                # chunk 0 is short: emit its out-proj eagerly so the PE has
                # work while chunk 1's x tiles finish landing
                drain((outproj_stream(0), pgen), 100)
                ogen = None
            else:
                drain((pgen,), 100)
                ogen = outproj_stream(c)
        drain((ogen,), 100)

    if not nc.is_finalized():
        nc.finalize()
    return nc


def _prep_core_inputs(inputs):
    """Build the 8 per-core input maps (host-side shard + transpose + cast)."""
    qx = np.asarray(inputs["q_input"], np.float32)
    kx = np.asarray(inputs["k_input"], np.float32)
    vx = np.asarray(inputs["v_input"], np.float32)
    W_q = np.asarray(inputs["W_q"], np.float32)
    W_k = np.asarray(inputs["W_k"], np.float32)
    W_v = np.asarray(inputs["W_v"], np.float32)
    W_o = np.asarray(inputs["W_o"], np.float32)
    for bn in ("b_q", "b_k", "b_v"):
        assert not np.any(np.asarray(inputs[bn])), f"nonzero {bn} unsupported"

    # RoPE column permutation: within each head, evens then odds
    perm = np.concatenate(
        [
            h * DK + np.concatenate([np.arange(0, DK, 2), np.arange(1, DK, 2)])
            for h in range(H)
        ]
    )
    W_q_p = W_q[:, perm]
    W_k_p = W_k[:, perm]

    # replicated trig tables for transposed-layout RoPE: [p, s], p%32 = pair idx
    theta = 10000.0 ** (-2.0 * np.arange(32, dtype=np.float64) / DK)
    pos = np.arange(S, dtype=np.float64)
    angT = theta[:, None] * pos[None, :]  # [32, S]
    cosT = np.tile(np.cos(angT), (4, 1)).astype(F16)  # [128, S]
    sinT = np.tile(np.sin(angT), (4, 1)).astype(F16)

    # signed even/odd swap matrix: rot = I@(ps*cos) + Msw@(ps*sin)
    msw = np.zeros((128, 128), np.float32)
    for h in range(2):
        for i in range(32):
            msw[h * 64 + 32 + i, h * 64 + i] = -1.0  # re -= o*sin
            msw[h * 64 + i, h * 64 + 32 + i] = 1.0  # ro += e*sin
    iden = np.eye(128, dtype=np.float32)
    tri = (np.arange(128)[None, :] >= np.arange(128)[:, None]).astype(np.float32)

    def shard_xT(x_b):  # [S, D] -> [128, NCH, MKT, 512] f16
        t = x_b.T.astype(F16).reshape(MKT, 128, NCH, 512)
        return t.transpose(1, 2, 0, 3).copy()

    in_maps = []
    for c in range(N_CORES):
        b = c // 4
        g = c % 4
        cols = slice(g * DC, (g + 1) * DC)
        m = {
            "xqT": shard_xT(qx[b]),
            "xkT": shard_xT(kx[b]),
            "xvT": shard_xT(vx[b]),
            "wqk": np.concatenate([W_q_p[:, cols], W_k_p[:, cols]], axis=1)
            .astype(F16)
            .reshape(MKT, 128, 2 * DC)
            .transpose(1, 0, 2)
            .copy(),
            "wv": W_v[:, cols].astype(F16).reshape(MKT, 128, DC).transpose(1, 0, 2).copy(),
            "wo": W_o[cols, :].astype(F16).reshape(2, 128, D).transpose(1, 0, 2).copy(),
            "cosd": cosT,
            "sind": sinT,
            "mswd": msw.astype(F16),
            "idend": iden.astype(F16),
            "trid": tri.astype(F16),
        }
        in_maps.append(m)
    return in_maps


def _check_causal(mask):
    mask = np.asarray(mask)
    jj = np.arange(S)
    assert np.array_equal(mask != 0, jj[None, :] <= jj[:, None]), (
        "only the causal mask is supported"
    )


def _run(inputs, trace=False, tmpdir=None, sim=False, sim_cores=(0,)):
    from concourse.bass_utils import run_bass_kernel_spmd

    _check_causal(inputs["mask"])
    in_maps = _prep_core_inputs(inputs)

    if "prog" not in _PROG_CACHE:
        _PROG_CACHE["prog"] = _build_program()
    nc = _PROG_CACHE["prog"]

    b_o = np.asarray(inputs["b_o"], np.float32)

    if sim:
        from concourse.bass_interp import CoreSim

        partials = {}
        for c in sim_cores:
            simr = CoreSim(nc)
            for name, val in in_maps[c].items():
                simr.tensor(name)[:] = val
            simr.simulate()
            partials[c] = np.array(simr.tensor("out"))
        return partials, None

    res = run_bass_kernel_spmd(
        nc, in_maps, list(range(N_CORES)), trace=trace, tmpdir=tmpdir
    )
    outs = [res.results[c]["out"].astype(np.float32) for c in range(N_CORES)]
    full = np.zeros((B, S, D), np.float32)
    for b in range(B):
        full[b] = outs[4 * b] + outs[4 * b + 1] + outs[4 * b + 2] + outs[4 * b + 3]
        full[b] += b_o[None, :]
    return full, res


def kernel(**inputs) -> np.ndarray:
    out, _ = _run(inputs, trace=False)
    return out


# revision 24
# speedup vs baseline: 1.0770x; 1.0707x over previous
"""Multi-head attention (RoPE + causal softmax) Bass kernel for 8 TRN2 cores.

Problem: B=2, S=2048, D=1024, H=16 heads, d_k=64.
Sharding: data-parallel over batch (2) x tensor-parallel over heads (4 groups
of 4 heads).  Core c handles batch c//4, heads [4*(c%4), 4*(c%4)+4).
Each core computes its heads' attention and a partial output projection
(W_o rows for its heads); the host sums the 4 partials per batch + b_o.

All operands fp16 (fp32 PSUM accumulation).  Per-core structure, emitted as
one software-pipelined stream so the PE never idles long enough to re-throttle:

  proj(c):  Q/K projected with stationary W slices -> psum arrives [d, q];
            RoPE as A=ps*cos, B=ps*sin (DVE) then rot = I@A + Msw@B on the
            PE (Msw is a signed even/odd swap matrix), evacuated fp16.
            V projected naturally (stationary X^T tiles), stored [k,h,65]
            with a ones column for the softmax denominator.
  attn(c):  per (pair, 2-k-tile group): transposed scores S^T[k,q] for both
            heads of the pair issued back-to-back as 64-row matmuls at
            tile_position (0,0)/(64,0) so they run concurrently; ScalarE
            exp (scale=1/8, 2x perf mode via fp16 out); diagonal blocks
            masked multiplicatively with a precomputed 0/1 triangle (DVE);
            PV with stationary V_ext=[V | 1s] accumulates ctx^T with the
            denominator in row 64.  Softmax division via ScalarE
            rec = exp(-ln(den)) (one table set has both Ln and Exp),
            GPSIMD partition-broadcast, one DVE multiply into ctxT fp16.
  out(c):   partial out = ctx @ W_o, fp16 to DRAM.

proj(c+1) is interleaved into attn(c) (one step per k-tile group) to keep
the tensor engine dense; out-proj of chunk c fills the gap before attn(c+1).

Softmax skips the max-subtraction: scores here are bounded (|s| < ~3) and
exp is computed in fp32, so softmax shift-invariance makes it exact enough.
"""

import sys

for _p in ("/opt/trn_rl_repo",):
    if _p not in sys.path:
        sys.path.insert(0, _p)

from contextlib import ExitStack

import numpy as np

import concourse.bass as bass
import concourse.mybir as mybir
import concourse.tile as tile
from concourse import bacc

F16 = np.float16

B = 2
S = 2048
D = 1024
H = 16
DK = 64
HPC = 4  # heads per core
DC = HPC * DK  # 256 model dims per core
N_CORES = 8
SCALE = 1.0 / np.sqrt(DK)
QT = S // 128  # 16 q tiles
KTILES = S // 128
MKT = 8  # model-dim k-tiles (1024/128)
NCH = 4  # 512-wide q chunks

_PROG_CACHE = {}


def _build_program():
    nc = bacc.Bacc("TRN2", target_bir_lowering=False, debug=False)
    f32 = mybir.dt.float32
    f16 = mybir.dt.float16

    # ---- DRAM I/O ----
    xqT = nc.dram_tensor("xqT", [128, NCH, MKT, 512], f16, kind="ExternalInput")
    xkT = nc.dram_tensor("xkT", [128, NCH, MKT, 512], f16, kind="ExternalInput")
    xvT = nc.dram_tensor("xvT", [128, NCH, MKT, 512], f16, kind="ExternalInput")
    wqk = nc.dram_tensor("wqk", [128, MKT, 2 * DC], f16, kind="ExternalInput")
    wv = nc.dram_tensor("wv", [128, MKT, DC], f16, kind="ExternalInput")
    wo = nc.dram_tensor("wo", [128, 2, D], f16, kind="ExternalInput")
    cosd = nc.dram_tensor("cosd", [128, S], f16, kind="ExternalInput")
    sind = nc.dram_tensor("sind", [128, S], f16, kind="ExternalInput")
    mswd = nc.dram_tensor("mswd", [128, 128], f16, kind="ExternalInput")
    idend = nc.dram_tensor("idend", [128, 128], f16, kind="ExternalInput")
    trid = nc.dram_tensor("trid", [128, 128], f16, kind="ExternalInput")
    out = nc.dram_tensor("out", [S, D], f16, kind="ExternalOutput")

    with tile.TileContext(nc) as tc, ExitStack() as top:
        persist = top.enter_context(tc.tile_pool(name="persist", bufs=1))
        px = top.enter_context(tc.tile_pool(name="px", bufs=2))
        pab = top.enter_context(tc.tile_pool(name="pab", bufs=2))
        pexp = top.enter_context(tc.tile_pool(name="pexp", bufs=2))
        pden = top.enter_context(tc.tile_pool(name="pden", bufs=2))
        pbc = top.enter_context(tc.tile_pool(name="pbc", bufs=2))
        posb = top.enter_context(tc.tile_pool(name="posb", bufs=4))
        pwork = top.enter_context(tc.tile_pool(name="pwork", bufs=2, space="PSUM"))
        psc = top.enter_context(tc.tile_pool(name="psc", bufs=1, space="PSUM"))
        pctx = top.enter_context(tc.tile_pool(name="pctx", bufs=1, space="PSUM"))

        # persistent SBUF tensors
        wv_sb = persist.tile([128, MKT, DC], f16, tag="wv")
        wqk_sb = persist.tile([128, MKT, 2 * DC], f16, tag="wqk")
        wo_sb = persist.tile([128, 2, D], f16, tag="wo")
        cos_sb = persist.tile([128, S], f16, tag="cos")
        sin_sb = persist.tile([128, S], f16, tag="sin")
        msw_sb = persist.tile([128, 128], f16, tag="msw")
        iden_sb = persist.tile([128, 128], f16, tag="iden")
        tri_sb = persist.tile([128, 128], f16, tag="tri")
        qtT = persist.tile([128, 2, QT, 128], f16, tag="qtT")
        ktT = persist.tile([128, 2, QT, 128], f16, tag="ktT")
        v_sb = persist.tile([128, KTILES, HPC, 65], f16, tag="v")
        ctxT = persist.tile([128, 2, QT, 128], f16, tag="ctxT")
        dum = persist.tile([1, 16], f32, tag="dum")
        warm = persist.tile([128, 512], f16, tag="warm")

        def load_chunk(c, eng=None):
            t = {}
            for key, dram in (("v", xvT), ("q", xqT), ("k", xkT)):
                xt = px.tile([128, MKT, 512], f16, tag=f"x{key}", name=f"x{key}_t")
                (eng or nc.sync).dma_start(xt[:], dram[:, c])
                t[key] = xt
            return t

        def alloc_chunk():
            return {
                key: px.tile([128, MKT, 512], f16, tag=f"x{key}", name=f"x{key}_t")
                for key in ("v", "q", "k")
            }

        # initial loads: one queue, ordered by when each tensor is first
        # needed (multi-queue spreading measured slower — the queues share
        # DMA resources and the split only added descriptor overhead)
        xts = {0: alloc_chunk(), 1: alloc_chunk()}
        nc.vector.memset(warm[:], 0.0)
        nc.vector.memset(dum[:], 1.0)
        nc.vector.memset(v_sb[:, :, :, 64:65], 1.0)
        x0, x1 = xts[0], xts[1]
        nc.sync.dma_start(wv_sb[:], wv[:])
        nc.sync.dma_start(x0["v"][:], xvT[:, 0])
        nc.sync.dma_start(wqk_sb[:], wqk[:])
        nc.sync.dma_start(cos_sb[:], cosd[:])
        nc.sync.dma_start(sin_sb[:], sind[:])
        nc.sync.dma_start(msw_sb[:], mswd[:])
        nc.sync.dma_start(iden_sb[:], idend[:])
        nc.sync.dma_start(x0["q"][:], xqT[:, 0])
        nc.sync.dma_start(x0["k"][:], xkT[:, 0])
        nc.sync.dma_start(tri_sb[:], trid[:])
        nc.sync.dma_start(x1["v"][:], xvT[:, 1])
        nc.sync.dma_start(wo_sb[:], wo[:])
        nc.sync.dma_start(x1["q"][:], xqT[:, 1])
        nc.sync.dma_start(x1["k"][:], xkT[:, 1])
        # preload the combined Ln+Exp table set during the initial DMAs; the
        # explicit load makes the insert_act_table_loads fixpoint see both
        # functions resident so it does not thrash between the Exp-only and
        # Ln-only sets on every softmax denominator.
        nc.scalar.add_instruction(
            mybir.InstLoadActFuncSet(
                name=nc.get_next_instruction_name(), act_func_set_id=6
            )
        )
        nc.scalar.activation(dum[:], dum[:], mybir.ActivationFunctionType.Exp)
        nc.scalar.activation(dum[:], dum[:], mybir.ActivationFunctionType.Ln)
        # dummy matmul stream over the initial DMA wait: sustained PE
        # activity flips the HAM clock gate to 8/8 before the real work
        wm_ps = pwork.tile([128, 512], f32, tag="w", name="wm_ps")
        for _ in range(48):
            nc.tensor.matmul(
                wm_ps[:], lhsT=warm[:, 0:128], rhs=warm[:], start=True, stop=True
            )

        def emit_rot(a_t, b_t, dstT, pair, c):
            rot = pwork.tile([128, 512], f32, tag="w", name="rot_ps")
            nc.tensor.matmul(rot[:], lhsT=iden_sb[:], rhs=a_t[:], start=True, stop=False)
            nc.tensor.matmul(rot[:], lhsT=msw_sb[:], rhs=b_t[:], start=False, stop=True)
            nc.vector.tensor_copy(
                dstT[:, pair, 4 * c : 4 * c + 4, :],
                rot[:].rearrange("p (t q) -> p t q", t=4),
            )

        def proj_stream(c, xt):
            csl = slice(c * 512, (c + 1) * 512)
            # V projection: natural layout, stationary X^T tiles
            for qq in range(4):
                v_ps = pwork.tile([128, 512], f32, tag="w", name="v_ps")
                for kt in range(MKT):
                    nc.tensor.matmul(
                        v_ps[:, 0:DC],
                        lhsT=xt["v"][:, kt, qq * 128 : (qq + 1) * 128],
                        rhs=wv_sb[:, kt, :],
                        start=(kt == 0),
                        stop=(kt == MKT - 1),
                    )
                nc.vector.tensor_copy(
                    v_sb[:, 4 * c + qq, :, 0:64],
                    v_ps[:, 0:DC].rearrange("p (h d) -> p h d", h=HPC),
                )
                yield
            # Q/K projections with RoPE (rot matmuls lag one iteration)
            pend = None
            for name_off, x_key, dstT in ((0, "q", qtT), (2 * 128, "k", ktT)):
                for pair in range(2):
                    wsl = slice(name_off + pair * 128, name_off + (pair + 1) * 128)
                    ps = pwork.tile([128, 512], f32, tag="w", name="pj_ps")
                    for kt in range(MKT):
                        nc.tensor.matmul(
                            ps[:],
                            lhsT=wqk_sb[:, kt, wsl],
                            rhs=xt[x_key][:, kt, :],
                            start=(kt == 0),
                            stop=(kt == MKT - 1),
                        )
                    a_t = pab.tile([128, 512], f16, tag="a", name="a_t")
                    nc.vector.tensor_mul(a_t[:], ps[:], cos_sb[:, csl])
                    b_t = pab.tile([128, 512], f16, tag="b", name="b_t")
                    nc.vector.tensor_mul(b_t[:], ps[:], sin_sb[:, csl])
                    if pend is not None:
                        emit_rot(*pend)
                    pend = (a_t, b_t, dstT, pair, c)
                    yield
            emit_rot(*pend)
            yield

        def drain(gens, n=1):
            for _ in range(n):
                for g in gens:
                    if g is None:
                        continue
                    try:
                        next(g)
                        break
                    except StopIteration:
                        continue

        def attn_pair(c, pair, gens):
            nk = 4 * (c + 1)
            qsl = slice(4 * c, 4 * c + 4)
            ctx2 = pctx.tile([65, 2, 512], f32, tag="ctx", name="ctx2")
            prev = None  # (kts, exps)
            for gp in range(nk // 2):
                kts = (2 * gp, 2 * gp + 1)
                diag = gp >= 2 * c
                # per-head score tiles: head A frees after exp_A, so the next
                # group's A-scores overlap exp_B (PE-bound steady state)
                scs = [
                    psc.tile([128, 2, 512], f32, tag=t, name=f"sc{t}")
                    for t in ("A", "B")
                ]
                exps = [
                    pexp.tile([128, 2, 512], f16, tag=t, name=f"e{t}")
                    for t in ("eA", "eB")
                ]
                # scores: full width (so exp reads fully-written tiles);
                # A/B adjacent for row-group concurrency
                for j, kt in enumerate(kts):
                    for hh in range(2):
                        d0 = hh * 64
                        nc.tensor.matmul(
                            scs[hh][:, j, :],
                            lhsT=ktT[d0 : d0 + 64, pair, kt, :],
                            rhs=qtT[d0 : d0 + 64, pair, qsl, :],
                            start=True,
                            stop=True,
                        )
                # PV of the previous group (software pipeline, lag 1)
                if prev is not None:
                    emit_pv(c, pair, ctx2, *prev, nk)
                for hh in range(2):
                    # exp full width (PV trims skip the sub-diagonal region)
                    nc.scalar.activation(
                        exps[hh][:],
                        scs[hh][:],
                        mybir.ActivationFunctionType.Exp,
                        scale=float(SCALE),
                    )
                if diag:
                    # multiplicative 0/1 triangle on the diagonal blocks
                    for hh in range(2):
                        for j, kt in enumerate(kts):
                            qo = kt - 4 * c
                            nc.vector.tensor_mul(
                                exps[hh][:, j, qo * 128 : (qo + 1) * 128],
                                exps[hh][:, j, qo * 128 : (qo + 1) * 128],
                                tri_sb[:],
                            )
                if gp % 2 == 1:
                    # ration the filler: keep some for the pair-boundary
                    # den/normalize chains where the PE would otherwise idle
                    drain(gens)
                prev = (kts, exps)
            emit_pv(c, pair, ctx2, *prev, nk)
            # denominators: rec = exp(-ln(den)); row 64 of the ctx psum
            lnt = pden.tile([1, 2, 512], f32, tag="ln", name="lnt")
            nc.scalar.activation(
                lnt[0:1, :, :], ctx2[64:65, :, :], mybir.ActivationFunctionType.Ln
            )
            rec = pden.tile([1, 2, 512], f32, tag="rec", name="rec")
            nc.scalar.activation(
                rec[:], lnt[:], mybir.ActivationFunctionType.Exp, scale=-1.0
            )
            for hh in range(2):
                bc = pbc.tile([64, 512], f32, tag=f"bc{hh}", name="bc")
                nc.gpsimd.partition_broadcast(bc[:], rec[0:1, hh, :])
                d0 = hh * 64
                nc.vector.tensor_mul(
                    ctxT[d0 : d0 + 64, pair, qsl, :],
                    ctx2[0:64, hh, :].rearrange("p (t q) -> p t q", t=4),
                    bc[:].rearrange("p (t q) -> p t q", t=4),
                )
            # queue PE filler over the serial den/normalize chain so the
            # tensor engine does not idle into a HAM re-throttle
            drain(gens, 3)

        def emit_pv(c, pair, ctx2, kts, exps, nk):
            for j, kt in enumerate(kts):
                qo = max(0, kt - 4 * c)
                for hh in range(2):
                    nc.tensor.matmul(
                        ctx2[:, hh, qo * 128 :],
                        lhsT=v_sb[:, kt, 2 * pair + hh, :],
                        rhs=exps[hh][:, j, qo * 128 :],
                        start=(kt == 0),
                        stop=(kt == nk - 1),
                    )

        def outproj_stream(c):
            for qt in range(4 * c, 4 * c + 4):
                for ec in range(2):
                    ops = pwork.tile([128, 512], f32, tag="w", name="o_ps")
                    for pair in range(2):
                        nc.tensor.matmul(
                            ops[:],
                            lhsT=ctxT[:, pair, qt, :],
                            rhs=wo_sb[:, pair, ec * 512 : (ec + 1) * 512],
                            start=(pair == 0),
                            stop=(pair == 1),
                        )
                    osb = posb.tile([128, 512], f16, tag="osb", name="osb")
                    nc.vector.tensor_copy(osb[:], ops[:])
                    nc.sync.dma_start(
                        out[qt * 128 : (qt + 1) * 128, ec * 512 : (ec + 1) * 512],
                        osb[:],
                    )
                    yield

        # ---------------- main emission ----------------
        for _ in proj_stream(0, xts[0]):
            pass
        ogens = []
        for c in range(NCH):
            if c + 2 < NCH:
                xts[c + 2] = load_chunk(c + 2)
            pgen = proj_stream(c + 1, xts[c + 1]) if c + 1 < NCH else None
            gens = tuple([pgen] + ogens)
            attn_pair(c, 0, gens)
            attn_pair(c, 1, gens)
            drain((pgen,), 1000)
            og = outproj_stream(c)
            if c == 0:
                # chunk 0 is short: emit its out-proj immediately (bridges
                # the wait for chunk 1's x tiles)
                drain((og,), 1000)
            else:
                # defer: later chunks' attention has less and less proj
                # filler, and the pair-boundary den chains need PE work
                ogens.append(og)
        drain(tuple(ogens), 1000)

    if not nc.is_finalized():
        nc.finalize()
    return nc


def _prep_core_inputs(inputs):
    """Build the 8 per-core input maps (host-side shard + transpose + cast)."""
    qx = np.asarray(inputs["q_input"], np.float32)
    kx = np.asarray(inputs["k_input"], np.float32)
    vx = np.asarray(inputs["v_input"], np.float32)
    W_q = np.asarray(inputs["W_q"], np.float32)
    W_k = np.asarray(inputs["W_k"], np.float32)
    W_v = np.asarray(inputs["W_v"], np.float32)
    W_o = np.asarray(inputs["W_o"], np.float32)
    for bn in ("b_q", "b_k", "b_v"):
        assert not np.any(np.asarray(inputs[bn])), f"nonzero {bn} unsupported"

    # RoPE column permutation: within each head, evens then odds
    perm = np.concatenate(
        [
            h * DK + np.concatenate([np.arange(0, DK, 2), np.arange(1, DK, 2)])
            for h in range(H)
        ]
    )
    W_q_p = W_q[:, perm]
    W_k_p = W_k[:, perm]

    # replicated trig tables for transposed-layout RoPE: [p, s], p%32 = pair idx
    theta = 10000.0 ** (-2.0 * np.arange(32, dtype=np.float64) / DK)
    pos = np.arange(S, dtype=np.float64)
    angT = theta[:, None] * pos[None, :]  # [32, S]
    cosT = np.tile(np.cos(angT), (4, 1)).astype(F16)  # [128, S]
    sinT = np.tile(np.sin(angT), (4, 1)).astype(F16)

    # signed even/odd swap matrix: rot = I@(ps*cos) + Msw@(ps*sin)
    msw = np.zeros((128, 128), np.float32)
    for h in range(2):
        for i in range(32):
            msw[h * 64 + 32 + i, h * 64 + i] = -1.0  # re -= o*sin
            msw[h * 64 + i, h * 64 + 32 + i] = 1.0  # ro += e*sin
    iden = np.eye(128, dtype=np.float32)
    tri = (np.arange(128)[None, :] >= np.arange(128)[:, None]).astype(np.float32)

    def shard_xT(x_b):  # [S, D] -> [128, NCH, MKT, 512] f16
        t = x_b.T.astype(F16).reshape(MKT, 128, NCH, 512)
        return t.transpose(1, 2, 0, 3).copy()

    in_maps = []
    for c in range(N_CORES):
        b = c // 4
        g = c % 4
        cols = slice(g * DC, (g + 1) * DC)
        m = {
            "xqT": shard_xT(qx[b]),
            "xkT": shard_xT(kx[b]),
            "xvT": shard_xT(vx[b]),
            "wqk": np.concatenate([W_q_p[:, cols], W_k_p[:, cols]], axis=1)
            .astype(F16)
            .reshape(MKT, 128, 2 * DC)
            .transpose(1, 0, 2)
            .copy(),
            "wv": W_v[:, cols].astype(F16).reshape(MKT, 128, DC).transpose(1, 0, 2).copy(),
            "wo": W_o[cols, :].astype(F16).reshape(2, 128, D).transpose(1, 0, 2).copy(),
            "cosd": cosT,
            "sind": sinT,
            "mswd": msw.astype(F16),
            "idend": iden.astype(F16),
            "trid": tri.astype(F16),
        }
        in_maps.append(m)
    return in_maps


def _check_causal(mask):
    mask = np.asarray(mask)
    jj = np.arange(S)
    assert np.array_equal(mask != 0, jj[None, :] <= jj[:, None]), (
        "only the causal mask is supported"
    )


def _run(inputs, trace=False, tmpdir=None, sim=False, sim_cores=(0,)):
    from concourse.bass_utils import run_bass_kernel_spmd

    _check_causal(inputs["mask"])
    in_maps = _prep_core_inputs(inputs)

    if "prog" not in _PROG_CACHE:
        _PROG_CACHE["prog"] = _build_program()
    nc = _PROG_CACHE["prog"]

    b_o = np.asarray(inputs["b_o"], np.float32)

    if sim:
        from concourse.bass_interp import CoreSim

        partials = {}
        for c in sim_cores:
            simr = CoreSim(nc)
            for name, val in in_maps[c].items():
                simr.tensor(name)[:] = val
            simr.simulate()
            partials[c] = np.array(simr.tensor("out"))
        return partials, None

    res = run_bass_kernel_spmd(
        nc, in_maps, list(range(N_CORES)), trace=trace, tmpdir=tmpdir
    )
    outs = [res.results[c]["out"].astype(np.float32) for c in range(N_CORES)]
    full = np.zeros((B, S, D), np.float32)
    for b in range(B):
        full[b] = outs[4 * b] + outs[4 * b + 1] + outs[4 * b + 2] + outs[4 * b + 3]
        full[b] += b_o[None, :]
    return full, res


def kernel(**inputs) -> np.ndarray:
    out, _ = _run(inputs, trace=False)
    return out
